# revision 4
# baseline (speedup 1.0000x reference)
"""Paged decode attention + cache update, distributed over 8 TRN2 NeuronCores.

Strategy (batch-parallel, 4 sequences/core):
- Host shards batch across cores; pre-transposes K pages to [D, slot] layout
  and packs V pages per (seq, kv-head) unit during sharding.
- Device per core: for each of 32 units (4 seqs x 8 kv heads), compute
  scoresT[slot, page*G+g] = KT_page.T @ qT (PE), add additive causal mask
  (DVE), exp (ACT, no max-subtraction -- scores have std ~1 so this is safe),
  V matmul accumulating over pages (PE), plus a ones-matmul for the softmax
  denominator (partition-dim reduction via PE).
- The new token's contribution (score vs itself) and the final normalization
  are rank-1 terms folded in on the host, as is the cache scatter-update.
"""

import numpy as np

B, H, HKV, D = 32, 32, 8, 128
P, S = 32, 128          # pages per sequence, slots per page
L = P * S
G = H // HKV            # GQA group = 4
NCORES = 8
BPC = B // NCORES       # sequences per core = 4
U = BPC * HKV           # units per core = 32
SCALE = 1.0 / np.sqrt(D)
NEG = -1e9

_COMPILED = {}


def _build():
    from contextlib import ExitStack

    import concourse.bass as bass
    import concourse.mybir as mybir
    import concourse.tile as tile
    from concourse import bacc

    f32 = mybir.dt.float32
    nc = bacc.Bacc()

    qt_d = nc.dram_tensor("qt", (D, U * G), f32, kind="ExternalInput")
    kt_d = nc.dram_tensor("kt", (U, D, L), f32, kind="ExternalInput")
    v_d = nc.dram_tensor("v", (U, S, P * D), f32, kind="ExternalInput")
    mask_d = nc.dram_tensor("mask", (BPC, S, P * G), f32, kind="ExternalInput")
    o_d = nc.dram_tensor("o", (U, D, G), f32, kind="ExternalOutput")
    den_d = nc.dram_tensor("den", (U, 1, P * G), f32, kind="ExternalOutput")

    with tile.TileContext(nc) as tc, ExitStack() as ctx:
        const = ctx.enter_context(tc.tile_pool(name="const", bufs=1))
        kpool = ctx.enter_context(tc.tile_pool(name="kpool", bufs=3))
        vpool = ctx.enter_context(tc.tile_pool(name="vpool", bufs=3))
        wpool = ctx.enter_context(tc.tile_pool(name="wpool", bufs=2))
        spool = ctx.enter_context(tc.tile_pool(name="spool", bufs=2))
        psc = ctx.enter_context(
            tc.tile_pool(name="psc", bufs=2, space=bass.MemorySpace.PSUM)
        )
        po = ctx.enter_context(
            tc.tile_pool(name="po", bufs=2, space=bass.MemorySpace.PSUM)
        )
        pd = ctx.enter_context(
            tc.tile_pool(name="pd", bufs=2, space=bass.MemorySpace.PSUM)
        )

        qt_t = const.tile([D, U * G], f32)
        nc.sync.dma_start(qt_t[:], qt_d[:])
        ones_t = const.tile([S, 1], f32)
        nc.gpsimd.memset(ones_t[:], 1.0)
        mask_ts = []
        for s in range(BPC):
            mt = const.tile([S, P * G], f32, tag=f"mask{s}")
            nc.sync.dma_start(mt[:], mask_d[s])
            mask_ts.append(mt)

        for u in range(U):
            s_idx = u // HKV
            kt_t = kpool.tile([D, L], f32)
            nc.sync.dma_start(kt_t[:], kt_d[u])
            v_t = vpool.tile([S, P * D], f32)
            nc.sync.dma_start(v_t[:], v_d[u])

            sc_ps = psc.tile([S, P * G], f32)
            for p in range(P):
                nc.tensor.matmul(
                    sc_ps[:, p * G : (p + 1) * G],
                    kt_t[:, p * S : (p + 1) * S],
                    qt_t[:, u * G : (u + 1) * G],
                    start=True,
                    stop=True,
                )
            sc_sb = spool.tile([S, P * G], f32, tag="sc")
            nc.vector.tensor_add(sc_sb[:], sc_ps[:], mask_ts[s_idx][:])
            w_t = wpool.tile([S, P * G], f32)
            nc.scalar.activation(w_t[:], sc_sb[:], mybir.ActivationFunctionType.Exp)

            den_ps = pd.tile([1, P * G], f32)
            nc.tensor.matmul(den_ps[:], ones_t[:], w_t[:], start=True, stop=True)

            o_ps = po.tile([D, G], f32)
            for p in range(P):
                nc.tensor.matmul(
                    o_ps[:],
                    v_t[:, p * D : (p + 1) * D],
                    w_t[:, p * G : (p + 1) * G],
                    start=(p == 0),
                    stop=(p == P - 1),
                )

            o_sb = spool.tile([D, G], f32, tag="osb")
            nc.vector.tensor_copy(o_sb[:], o_ps[:])
            den_sb = spool.tile([1, P * G], f32, tag="densb")
            nc.vector.tensor_copy(den_sb[:], den_ps[:])
            nc.sync.dma_start(o_d[u], o_sb[:])
            nc.sync.dma_start(den_d[u], den_sb[:])

    nc.compile()
    return nc


def _get_nc():
    if "nc" not in _COMPILED:
        _COMPILED["nc"] = _build()
    return _COMPILED["nc"]


def kernel(query, key, value, k_cache, v_cache, cache_position, page_table):
    from concourse.bass_utils import run_bass_kernel_spmd

    query = np.asarray(query, dtype=np.float32)
    key = np.asarray(key, dtype=np.float32)
    value = np.asarray(value, dtype=np.float32)
    k_cache = np.asarray(k_cache, dtype=np.float32)
    v_cache = np.asarray(v_cache, dtype=np.float32)
    pos = np.asarray(cache_position, dtype=np.int64)
    pt = np.asarray(page_table, dtype=np.int64)

    nc = _get_nc()

    qg = query.reshape(B, HKV, G, D)
    l_grid = (np.arange(P)[:, None] * S + np.arange(S)[None, :])  # [P, S]

    in_maps = []
    for i in range(NCORES):
        seqs = np.arange(i * BPC, (i + 1) * BPC)
        # gather pages: [s, p, hkv, slot, d]
        kg = k_cache[pt[seqs]]
        vg = v_cache[pt[seqs]]
        kt = np.ascontiguousarray(kg.transpose(0, 2, 4, 1, 3)).reshape(U, D, L)
        vv = np.ascontiguousarray(vg.transpose(0, 2, 3, 1, 4)).reshape(U, S, P * D)
        qt = np.ascontiguousarray(
            (qg[seqs] * SCALE).transpose(3, 0, 1, 2)
        ).reshape(D, U * G)
        # additive mask: cache position l valid iff l < pos (slot pos itself is
        # the new token, folded in on the host)
        valid = l_grid[None] < pos[seqs][:, None, None]          # [s, P, S]
        m = np.where(valid, 0.0, NEG).astype(np.float32)         # [s, P, S]
        mask = np.ascontiguousarray(
            np.broadcast_to(m.transpose(0, 2, 1)[:, :, :, None], (BPC, S, P, G))
        ).reshape(BPC, S, P * G)
        in_maps.append({"qt": qt, "kt": kt, "v": vv, "mask": mask})

    _COMPILED["in_maps"] = in_maps
    res = run_bass_kernel_spmd(nc, in_maps, core_ids=list(range(NCORES)))
    outs = res.results

    o = np.stack([outs[i]["o"] for i in range(NCORES)])        # [NC, U, D, G]
    den = np.stack([outs[i]["den"] for i in range(NCORES)])    # [NC, U, 1, P*G]

    # [NC, U, D, G] -> [B, HKV, G, D]
    o = o.reshape(NCORES, BPC, HKV, D, G).transpose(0, 1, 2, 4, 3).reshape(
        B, HKV, G, D
    )
    den_sum = (
        den.reshape(NCORES, BPC, HKV, P, G).sum(3).reshape(B, HKV, G)
    )

    # new-token contribution (host rank-1 term)
    s_new = np.einsum("bkgd,bkd->bkg", qg, key[:, :, 0, :]) * SCALE
    w_new = np.exp(s_new)                                       # [B, HKV, G]
    num = o + w_new[..., None] * value[:, :, 0, :][:, :, None, :]
    out = (num / (den_sum + w_new)[..., None]).reshape(B, H, 1, D)

    # cache update (host scatter)
    kc = np.array(k_cache)
    vc = np.array(v_cache)
    phys = pt[np.arange(B), pos // S]
    slot = pos % S
    kc[phys, :, slot, :] = key[:, :, 0, :]
    vc[phys, :, slot, :] = value[:, :, 0, :]

    return out.astype(np.float32), kc, vc


# revision 6
# speedup vs baseline: 2.9511x; 2.9511x over previous
"""Paged decode attention + cache update, distributed over 8 TRN2 NeuronCores.

Strategy (batch-parallel, 4 sequences/core, bf16 compute):
- Host shards batch across cores; pre-transposes K pages to [D, slot] layout
  and packs V pages per (seq, kv-head) unit during sharding; casts K/V/q bf16.
- Device per core: for each of 32 units (4 seqs x 8 kv heads), compute
  scoresT[slot, page*G+g] = KT_page.T @ qT (PE), add additive causal mask
  (DVE), exp (ACT, no max-subtraction -- scores have std ~1 so this is safe),
  V matmul accumulating over pages (PE), plus a ones-matmul for the softmax
  denominator (partition-dim reduction via PE).
- The new token's contribution (score vs itself) and the final normalization
  are rank-1 terms folded in on the host, as is the cache scatter-update.
"""

import numpy as np

B, H, HKV, D = 32, 32, 8, 128
P, S = 32, 128          # pages per sequence, slots per page
L = P * S
G = H // HKV            # GQA group = 4
NCORES = 8
BPC = B // NCORES       # sequences per core = 4
U = BPC * HKV           # units per core = 32
SCALE = 1.0 / np.sqrt(D)
NEG = -1e9

_COMPILED = {}


def _build():
    from contextlib import ExitStack

    import concourse.bass as bass
    import concourse.mybir as mybir
    import concourse.tile as tile
    from concourse import bacc

    f32 = mybir.dt.float32
    bf16 = mybir.dt.bfloat16
    nc = bacc.Bacc()

    qt_d = nc.dram_tensor("qt", (D, U * G), bf16, kind="ExternalInput")
    kt_d = nc.dram_tensor("kt", (U, D, L), bf16, kind="ExternalInput")
    v_d = nc.dram_tensor("v", (U, S, P * D), bf16, kind="ExternalInput")
    mask_d = nc.dram_tensor("mask", (BPC, S, P * G), f32, kind="ExternalInput")
    o_d = nc.dram_tensor("o", (U, D, G), f32, kind="ExternalOutput")
    den_d = nc.dram_tensor("den", (U, 1, P * G), f32, kind="ExternalOutput")

    with tile.TileContext(nc) as tc, ExitStack() as ctx:
        const = ctx.enter_context(tc.tile_pool(name="const", bufs=1))
        kpool = ctx.enter_context(tc.tile_pool(name="kpool", bufs=4))
        vpool = ctx.enter_context(tc.tile_pool(name="vpool", bufs=4))
        wpool = ctx.enter_context(tc.tile_pool(name="wpool", bufs=2))
        spool = ctx.enter_context(tc.tile_pool(name="spool", bufs=2))
        psc = ctx.enter_context(
            tc.tile_pool(name="psc", bufs=2, space=bass.MemorySpace.PSUM)
        )
        po = ctx.enter_context(
            tc.tile_pool(name="po", bufs=2, space=bass.MemorySpace.PSUM)
        )
        pd = ctx.enter_context(
            tc.tile_pool(name="pd", bufs=2, space=bass.MemorySpace.PSUM)
        )

        qt_t = const.tile([D, U * G], bf16)
        nc.sync.dma_start(qt_t[:], qt_d[:])
        ones_t = const.tile([S, 1], bf16)
        nc.gpsimd.memset(ones_t[:], 1.0)
        mask_ts = []
        for s in range(BPC):
            mt = const.tile([S, P * G], f32, tag=f"mask{s}")
            nc.sync.dma_start(mt[:], mask_d[s])
            mask_ts.append(mt)

        for u in range(U):
            s_idx = u // HKV
            kt_t = kpool.tile([D, L], bf16)
            nc.sync.dma_start(kt_t[:], kt_d[u])
            v_t = vpool.tile([S, P * D], bf16)
            nc.sync.dma_start(v_t[:], v_d[u])

            sc_ps = psc.tile([S, P * G], f32)
            for p in range(P):
                nc.tensor.matmul(
                    sc_ps[:, p * G : (p + 1) * G],
                    kt_t[:, p * S : (p + 1) * S],
                    qt_t[:, u * G : (u + 1) * G],
                    start=True,
                    stop=True,
                )
            sc_sb = spool.tile([S, P * G], f32, tag="sc")
            nc.vector.tensor_add(sc_sb[:], sc_ps[:], mask_ts[s_idx][:])
            w_t = wpool.tile([S, P * G], bf16)
            nc.scalar.activation(w_t[:], sc_sb[:], mybir.ActivationFunctionType.Exp)

            den_ps = pd.tile([1, P * G], f32)
            nc.tensor.matmul(den_ps[:], ones_t[:], w_t[:], start=True, stop=True)

            o_ps = po.tile([D, G], f32)
            for p in range(P):
                nc.tensor.matmul(
                    o_ps[:],
                    v_t[:, p * D : (p + 1) * D],
                    w_t[:, p * G : (p + 1) * G],
                    start=(p == 0),
                    stop=(p == P - 1),
                )

            o_sb = spool.tile([D, G], f32, tag="osb")
            nc.vector.tensor_copy(o_sb[:], o_ps[:])
            den_sb = spool.tile([1, P * G], f32, tag="densb")
            nc.vector.tensor_copy(den_sb[:], den_ps[:])
            nc.sync.dma_start(o_d[u], o_sb[:])
            nc.sync.dma_start(den_d[u], den_sb[:])

    nc.compile()
    return nc


def _get_nc():
    if "nc" not in _COMPILED:
        _COMPILED["nc"] = _build()
    return _COMPILED["nc"]


def kernel(query, key, value, k_cache, v_cache, cache_position, page_table):
    import ml_dtypes

    from concourse.bass_utils import run_bass_kernel_spmd

    bf16 = ml_dtypes.bfloat16
    query = np.asarray(query, dtype=np.float32)
    key = np.asarray(key, dtype=np.float32)
    value = np.asarray(value, dtype=np.float32)
    k_cache = np.asarray(k_cache, dtype=np.float32)
    v_cache = np.asarray(v_cache, dtype=np.float32)
    pos = np.asarray(cache_position, dtype=np.int64)
    pt = np.asarray(page_table, dtype=np.int64)

    nc = _get_nc()

    qg = query.reshape(B, HKV, G, D)
    l_grid = (np.arange(P)[:, None] * S + np.arange(S)[None, :])  # [P, S]

    in_maps = []
    for i in range(NCORES):
        seqs = np.arange(i * BPC, (i + 1) * BPC)
        # gather pages: [s, p, hkv, slot, d]
        kg = k_cache[pt[seqs]]
        vg = v_cache[pt[seqs]]
        kt = np.ascontiguousarray(
            kg.transpose(0, 2, 4, 1, 3), dtype=bf16
        ).reshape(U, D, L)
        vv = np.ascontiguousarray(
            vg.transpose(0, 2, 3, 1, 4), dtype=bf16
        ).reshape(U, S, P * D)
        qt = np.ascontiguousarray(
            (qg[seqs] * SCALE).transpose(3, 0, 1, 2), dtype=bf16
        ).reshape(D, U * G)
        # additive mask: cache position l valid iff l < pos (slot pos itself is
        # the new token, folded in on the host)
        valid = l_grid[None] < pos[seqs][:, None, None]          # [s, P, S]
        m = np.where(valid, 0.0, NEG).astype(np.float32)         # [s, P, S]
        mask = np.ascontiguousarray(
            np.broadcast_to(m.transpose(0, 2, 1)[:, :, :, None], (BPC, S, P, G))
        ).reshape(BPC, S, P * G)
        in_maps.append({"qt": qt, "kt": kt, "v": vv, "mask": mask})

    _COMPILED["in_maps"] = in_maps
    res = run_bass_kernel_spmd(nc, in_maps, core_ids=list(range(NCORES)))
    outs = res.results

    o = np.stack([outs[i]["o"] for i in range(NCORES)])        # [NC, U, D, G]
    den = np.stack([outs[i]["den"] for i in range(NCORES)])    # [NC, U, 1, P*G]

    # [NC, U, D, G] -> [B, HKV, G, D]
    o = o.reshape(NCORES, BPC, HKV, D, G).transpose(0, 1, 2, 4, 3).reshape(
        B, HKV, G, D
    )
    den_sum = (
        den.reshape(NCORES, BPC, HKV, P, G).sum(3).reshape(B, HKV, G)
    )

    # new-token contribution (host rank-1 term)
    s_new = np.einsum("bkgd,bkd->bkg", qg, key[:, :, 0, :]) * SCALE
    w_new = np.exp(s_new)                                       # [B, HKV, G]
    num = o + w_new[..., None] * value[:, :, 0, :][:, :, None, :]
    out = (num / (den_sum + w_new)[..., None]).reshape(B, H, 1, D)

    # cache update (host scatter)
    kc = np.array(k_cache)
    vc = np.array(v_cache)
    phys = pt[np.arange(B), pos // S]
    slot = pos % S
    kc[phys, :, slot, :] = key[:, :, 0, :]
    vc[phys, :, slot, :] = value[:, :, 0, :]

    return out.astype(np.float32), kc, vc


# revision 8
# speedup vs baseline: 4.6102x; 1.5622x over previous
"""Paged decode attention + cache update, distributed over 8 TRN2 NeuronCores.

Strategy (unit-parallel with truncation, bf16 compute):
- A unit = one (sequence, kv-head) pair; 256 units total. Units are sorted by
  sequence length and dealt round-robin across the 8 cores, so graph slot j
  holds 8 near-equal-length units and gets a static page budget
  budget[j] = max of those 8 lengths. Only pages below each sequence's cache
  position are shipped/computed (~57% of the full cache here).
- Host packs, per core, the K pages transposed to [D, slot] and V pages
  native, both bf16, plus additive causal masks; all as flat arrays with
  static per-slot offsets (the same graph runs on all 8 cores).
- Device per core, per unit slot j (budget n): n score matmuls
  (lhsT = KT page, rhs = qT[:, 4j:4j+4]) -> PSUM scoresT [slot, p*G+g];
  DVE mask-add; ACT exp (no max-subtraction -- scores have std ~1);
  ones-matmul for the softmax denominator; n V matmuls accumulating
  out [D, G] over pages.
- The new token's contribution and final normalization are rank-1 terms
  folded in on the host, as is the cache scatter-update.
"""

import numpy as np

B, H, HKV, D = 32, 32, 8, 128
P, S = 32, 128          # pages per sequence, slots per page
L = P * S
G = H // HKV            # GQA group = 4
NCORES = 8
U = 32                  # unit slots per core (B*HKV/NCORES)
SCALE = 1.0 / np.sqrt(D)
NEG = -1e9

_COMPILED = {}


def _plan(pos):
    """Static schedule from cache positions: per-slot budgets + unit deal."""
    n_pages = -(-pos // S)              # valid cache pages per sequence
    units = np.repeat(n_pages, HKV)     # unit id = b*HKV + h
    order = np.argsort(-units, kind="stable")
    budgets = tuple(int(units[order[8 * j]]) for j in range(U))
    # core i, slot j <- unit order[8j + i]
    assign = order.reshape(U, NCORES)
    offs = np.concatenate([[0], np.cumsum(budgets)]).astype(np.int64)
    return budgets, assign, offs


def _build(budgets):
    from contextlib import ExitStack

    import concourse.bass as bass
    import concourse.mybir as mybir
    import concourse.tile as tile
    from concourse import bacc

    f32 = mybir.dt.float32
    bf16 = mybir.dt.bfloat16
    nc = bacc.Bacc()
    tot = sum(budgets)

    qt_d = nc.dram_tensor("qt", (D, U * G), bf16, kind="ExternalInput")
    kt_d = nc.dram_tensor("kt", (tot * D * S,), bf16, kind="ExternalInput")
    v_d = nc.dram_tensor("v", (tot * S * D,), bf16, kind="ExternalInput")
    mask_d = nc.dram_tensor(
        "mask", (tot * S * G,), f32, kind="ExternalInput"
    )
    o_d = nc.dram_tensor("o", (U, D, G), f32, kind="ExternalOutput")
    den_d = nc.dram_tensor("den", (U, 1, P * G), f32, kind="ExternalOutput")

    with tile.TileContext(nc) as tc, ExitStack() as ctx:
        const = ctx.enter_context(tc.tile_pool(name="const", bufs=1))
        kpool = ctx.enter_context(tc.tile_pool(name="kpool", bufs=6))
        vpool = ctx.enter_context(tc.tile_pool(name="vpool", bufs=6))
        mpool = ctx.enter_context(tc.tile_pool(name="mpool", bufs=4))
        wpool = ctx.enter_context(tc.tile_pool(name="wpool", bufs=2))
        spool = ctx.enter_context(tc.tile_pool(name="spool", bufs=2))
        psc = ctx.enter_context(
            tc.tile_pool(name="psc", bufs=2, space=bass.MemorySpace.PSUM)
        )
        po = ctx.enter_context(
            tc.tile_pool(name="po", bufs=2, space=bass.MemorySpace.PSUM)
        )
        pd = ctx.enter_context(
            tc.tile_pool(name="pd", bufs=2, space=bass.MemorySpace.PSUM)
        )

        qt_t = const.tile([D, U * G], bf16)
        nc.sync.dma_start(qt_t[:], qt_d[:])
        ones_t = const.tile([S, 1], bf16)
        nc.gpsimd.memset(ones_t[:], 1.0)

        off = 0
        for j in range(U):
            n = budgets[j]
            if n == 0:
                continue
            kt_t = kpool.tile([D, n * S], bf16, tag="kt")
            nc.sync.dma_start(
                kt_t[:],
                kt_d[off * D * S : (off + n) * D * S].rearrange(
                    "(d f) -> d f", d=D
                ),
            )
            v_t = vpool.tile([S, n * D], bf16, tag="vt")
            nc.sync.dma_start(
                v_t[:],
                v_d[off * S * D : (off + n) * S * D].rearrange(
                    "(s f) -> s f", s=S
                ),
            )
            m_t = mpool.tile([S, n * G], f32, tag="mask")
            nc.sync.dma_start(
                m_t[:],
                mask_d[off * S * G : (off + n) * S * G].rearrange(
                    "(s f) -> s f", s=S
                ),
            )

            sc_ps = psc.tile([S, n * G], f32, tag="sc")
            for p in range(n):
                nc.tensor.matmul(
                    sc_ps[:, p * G : (p + 1) * G],
                    kt_t[:, p * S : (p + 1) * S],
                    qt_t[:, j * G : (j + 1) * G],
                    start=True,
                    stop=True,
                )
            sc_sb = spool.tile([S, n * G], f32, tag="sc_sb")
            nc.vector.tensor_add(sc_sb[:], sc_ps[:], m_t[:])
            w_t = wpool.tile([S, n * G], bf16, tag="w")
            nc.scalar.activation(
                w_t[:], sc_sb[:], mybir.ActivationFunctionType.Exp
            )

            den_ps = pd.tile([1, n * G], f32, tag="den")
            nc.tensor.matmul(den_ps[:], ones_t[:], w_t[:], start=True, stop=True)

            o_ps = po.tile([D, G], f32, tag="o")
            for p in range(n):
                nc.tensor.matmul(
                    o_ps[:],
                    v_t[:, p * D : (p + 1) * D],
                    w_t[:, p * G : (p + 1) * G],
                    start=(p == 0),
                    stop=(p == n - 1),
                )

            o_sb = spool.tile([D, G], f32, tag="osb")
            nc.vector.tensor_copy(o_sb[:], o_ps[:])
            den_sb = spool.tile([1, n * G], f32, tag="densb")
            nc.vector.tensor_copy(den_sb[:], den_ps[:])
            nc.sync.dma_start(o_d[j], o_sb[:])
            nc.sync.dma_start(den_d[j, :, : n * G], den_sb[:])
            off += n

    nc.compile()
    return nc


def _get_nc(budgets):
    if budgets not in _COMPILED:
        _COMPILED[budgets] = _build(budgets)
    return _COMPILED[budgets]


def kernel(query, key, value, k_cache, v_cache, cache_position, page_table):
    import ml_dtypes

    from concourse.bass_utils import run_bass_kernel_spmd

    bf16 = ml_dtypes.bfloat16
    query = np.asarray(query, dtype=np.float32)
    key = np.asarray(key, dtype=np.float32)
    value = np.asarray(value, dtype=np.float32)
    k_cache = np.asarray(k_cache, dtype=np.float32)
    v_cache = np.asarray(v_cache, dtype=np.float32)
    pos = np.asarray(cache_position, dtype=np.int64)
    pt = np.asarray(page_table, dtype=np.int64)

    budgets, assign, offs = _plan(pos)
    tot = int(sum(budgets))
    nc = _get_nc(budgets)
    _COMPILED["last_nc"] = nc

    qg = (query.reshape(B, HKV, G, D) * SCALE).astype(bf16)
    slot_idx = np.arange(S)

    in_maps = []
    for i in range(NCORES):
        kt = np.zeros(tot * D * S, dtype=bf16)
        vv = np.zeros(tot * S * D, dtype=bf16)
        mask = np.full(tot * S * G, NEG, dtype=np.float32)
        qt = np.zeros((D, U * G), dtype=bf16)
        for j in range(U):
            nb = budgets[j]
            if nb == 0:
                continue
            uid = assign[j, i]
            b, h = divmod(int(uid), HKV)
            nv = int(-(-pos[b] // S))            # valid pages for this seq
            pages = pt[b][:nv]
            o0 = int(offs[j])
            kj = k_cache[pages, h].astype(bf16)  # [nv, S, D]
            vj = v_cache[pages, h].astype(bf16)
            ktj = np.zeros((D, nb * S), dtype=bf16)
            ktj[:, : nv * S] = kj.transpose(2, 0, 1).reshape(D, nv * S)
            vvj = np.zeros((S, nb * D), dtype=bf16)
            vvj[:, : nv * D] = vj.transpose(1, 0, 2).reshape(S, nv * D)
            kt[o0 * D * S : (o0 + nb) * D * S] = ktj.reshape(-1)
            vv[o0 * S * D : (o0 + nb) * S * D] = vvj.reshape(-1)
            # mask[s, p*G+g] additive: l = p*S+slot valid iff l < pos[b]
            l_g = np.arange(nb)[None, :] * S + slot_idx[:, None]  # [S, nb]
            mj = np.where(l_g < pos[b], 0.0, NEG).astype(np.float32)
            mask[o0 * S * G : (o0 + nb) * S * G] = np.repeat(
                mj, G, axis=1
            ).reshape(-1)
            qt[:, j * G : (j + 1) * G] = qg[b, h].T
        in_maps.append({"qt": qt, "kt": kt, "v": vv, "mask": mask})

    _COMPILED["in_maps"] = in_maps
    res = run_bass_kernel_spmd(nc, in_maps, core_ids=list(range(NCORES)))
    outs = res.results

    out_bhg = np.zeros((B, HKV, G, D), dtype=np.float64)
    den_sum = np.zeros((B, HKV, G), dtype=np.float64)
    for i in range(NCORES):
        o = outs[i]["o"]          # [U, D, G]
        den = outs[i]["den"]      # [U, 1, P*G]
        for j in range(U):
            nb = budgets[j]
            uid = assign[j, i]
            b, h = divmod(int(uid), HKV)
            if nb == 0:
                continue
            out_bhg[b, h] = o[j].T
            den_sum[b, h] = den[j, 0, : nb * G].reshape(nb, G).sum(0)

    # new-token contribution (host rank-1 term)
    qgf = query.reshape(B, HKV, G, D)
    s_new = np.einsum("bkgd,bkd->bkg", qgf, key[:, :, 0, :]) * SCALE
    w_new = np.exp(s_new)                                       # [B, HKV, G]
    num = out_bhg + w_new[..., None] * value[:, :, 0, :][:, :, None, :]
    out = (num / (den_sum + w_new)[..., None]).reshape(B, H, 1, D)

    # cache update (host scatter)
    kc = np.array(k_cache)
    vc = np.array(v_cache)
    phys = pt[np.arange(B), pos // S]
    slot = pos % S
    kc[phys, :, slot, :] = key[:, :, 0, :]
    vc[phys, :, slot, :] = value[:, :, 0, :]

    return out.astype(np.float32), kc, vc


# revision 9
# speedup vs baseline: 8.6542x; 1.8772x over previous
"""Paged decode attention + cache update, distributed over 8 TRN2 NeuronCores.

Strategy (unit-parallel with truncation, bf16 compute):
- A unit = one (sequence, kv-head) pair; 256 units total. Units are sorted by
  sequence length and dealt round-robin across the 8 cores, so graph slot j
  holds 8 near-equal-length units and gets a static page budget
  budget[j] = max of those 8 lengths. Only pages below each sequence's cache
  position are shipped/computed (~57% of the full cache here).
- Host packs, per core, the K pages transposed to [D, slot] and V pages
  native, both bf16, as flat arrays with static per-slot offsets (the same
  graph runs SPMD on all 8 cores). K loads issue on the Sync HWDGE, V loads
  on the Scalar HWDGE to keep both DMA issue streams fed.
- Device per core, per unit slot j (budget n): n score matmuls
  (lhsT = KT page, rhs = qT[:, 4j:4j+4]) -> PSUM scoresT [slot, p*G+g];
  ACT exp straight from PSUM (no max-subtraction -- scores have std ~1);
  causal masking is applied multiplicatively AFTER exp with an on-device
  iota-vs-position compare (saves all mask DMA traffic); ones-matmul gives
  the softmax denominator; n V matmuls accumulate out [D, G] over pages.
- The new token's contribution and final normalization are rank-1 terms
  folded in on the host, as is the cache scatter-update.
"""

import numpy as np

B, H, HKV, D = 32, 32, 8, 128
P, S = 32, 128          # pages per sequence, slots per page
L = P * S
G = H // HKV            # GQA group = 4
NCORES = 8
U = 32                  # unit slots per core (B*HKV/NCORES)
SCALE = 1.0 / np.sqrt(D)
NEG = -1e9

_COMPILED = {}


def _plan(pos):
    """Static schedule from cache positions: per-slot budgets + unit deal."""
    n_pages = -(-pos // S)              # valid cache pages per sequence
    units = np.repeat(n_pages, HKV)     # unit id = b*HKV + h
    order = np.argsort(-units, kind="stable")
    budgets = tuple(int(units[order[8 * j]]) for j in range(U))
    assign = order.reshape(U, NCORES)   # core i, slot j <- unit assign[j, i]
    offs = np.concatenate([[0], np.cumsum(budgets)]).astype(np.int64)
    return budgets, assign, offs


def _build(budgets):
    from contextlib import ExitStack

    import concourse.bass as bass
    import concourse.mybir as mybir
    import concourse.tile as tile
    from concourse import bacc

    f32 = mybir.dt.float32
    bf16 = mybir.dt.bfloat16
    i32 = mybir.dt.int32
    nc = bacc.Bacc()
    tot = sum(budgets)
    dtot = sum(n * G for n in budgets)

    qt_d = nc.dram_tensor("qt", (D, U * G), bf16, kind="ExternalInput")
    kt_d = nc.dram_tensor("kt", (tot * D * S,), bf16, kind="ExternalInput")
    v_d = nc.dram_tensor("v", (tot * S * D,), bf16, kind="ExternalInput")
    pos_d = nc.dram_tensor("posv", (S, U), f32, kind="ExternalInput")
    o_d = nc.dram_tensor("o", (D, U * G), f32, kind="ExternalOutput")
    den_d = nc.dram_tensor("den", (1, dtot), f32, kind="ExternalOutput")

    with tile.TileContext(nc) as tc, ExitStack() as ctx:
        const = ctx.enter_context(tc.tile_pool(name="const", bufs=1))
        kpool = ctx.enter_context(tc.tile_pool(name="kpool", bufs=8))
        vpool = ctx.enter_context(tc.tile_pool(name="vpool", bufs=8))
        wpool = ctx.enter_context(tc.tile_pool(name="wpool", bufs=3))
        spool = ctx.enter_context(tc.tile_pool(name="spool", bufs=3))
        psc = ctx.enter_context(
            tc.tile_pool(name="psc", bufs=4, space=bass.MemorySpace.PSUM)
        )
        po = ctx.enter_context(
            tc.tile_pool(name="po", bufs=2, space=bass.MemorySpace.PSUM)
        )
        pd = ctx.enter_context(
            tc.tile_pool(name="pd", bufs=2, space=bass.MemorySpace.PSUM)
        )

        qt_t = const.tile([D, U * G], bf16)
        nc.gpsimd.dma_start(qt_t[:], qt_d[:])
        pos_t = const.tile([S, U], f32)
        nc.gpsimd.dma_start(pos_t[:], pos_d[:])
        ones_t = const.tile([S, 1], bf16)
        nc.gpsimd.memset(ones_t[:], 1.0)
        # iota[s, p*G+g] = p*S + s (logical cache position of that column)
        iota_i = const.tile([S, P * G], i32)
        nc.gpsimd.iota(
            iota_i[:], [[S, P], [0, G]], channel_multiplier=1
        )
        iota_f = const.tile([S, P * G], f32)
        nc.vector.tensor_copy(iota_f[:], iota_i[:])

        o_all = const.tile([D, U * G], f32, tag="o_all")
        den_all = const.tile([1, dtot], f32, tag="den_all")

        off = 0
        doff = 0
        for j in range(U):
            n = budgets[j]
            if n == 0:
                continue
            kt_t = kpool.tile([D, n * S], bf16, tag="kt")
            nc.sync.dma_start(
                kt_t[:],
                kt_d[off * D * S : (off + n) * D * S].rearrange(
                    "(d f) -> d f", d=D
                ),
            )
            v_t = vpool.tile([S, n * D], bf16, tag="vt")
            nc.scalar.dma_start(
                v_t[:],
                v_d[off * S * D : (off + n) * S * D].rearrange(
                    "(s f) -> s f", s=S
                ),
            )

            sc_ps = psc.tile([S, n * G], f32, tag="sc")
            for p in range(n):
                nc.tensor.matmul(
                    sc_ps[:, p * G : (p + 1) * G],
                    kt_t[:, p * S : (p + 1) * S],
                    qt_t[:, j * G : (j + 1) * G],
                    start=True,
                    stop=True,
                )
            w_raw = wpool.tile([S, n * G], bf16, tag="w_raw")
            nc.scalar.activation(
                w_raw[:], sc_ps[:], mybir.ActivationFunctionType.Exp
            )
            m_t = spool.tile([S, n * G], bf16, tag="m")
            nc.vector.tensor_scalar(
                m_t[:],
                iota_f[:, : n * G],
                pos_t[:, j : j + 1],
                None,
                mybir.AluOpType.is_lt,
            )
            w_t = wpool.tile([S, n * G], bf16, tag="w")
            nc.vector.tensor_mul(w_t[:], w_raw[:], m_t[:])

            den_ps = pd.tile([1, n * G], f32, tag="den")
            nc.tensor.matmul(den_ps[:], ones_t[:], w_t[:], start=True, stop=True)

            o_ps = po.tile([D, G], f32, tag="o")
            for p in range(n):
                nc.tensor.matmul(
                    o_ps[:],
                    v_t[:, p * D : (p + 1) * D],
                    w_t[:, p * G : (p + 1) * G],
                    start=(p == 0),
                    stop=(p == n - 1),
                )

            nc.vector.tensor_copy(o_all[:, j * G : (j + 1) * G], o_ps[:])
            nc.vector.tensor_copy(
                den_all[:, doff : doff + n * G], den_ps[:]
            )
            off += n
            doff += n * G

        nc.gpsimd.dma_start(o_d[:], o_all[:])
        nc.gpsimd.dma_start(den_d[:], den_all[:])

    nc.compile()
    return nc


def _get_nc(budgets):
    if budgets not in _COMPILED:
        _COMPILED[budgets] = _build(budgets)
    return _COMPILED[budgets]


def kernel(query, key, value, k_cache, v_cache, cache_position, page_table):
    import ml_dtypes

    from concourse.bass_utils import run_bass_kernel_spmd

    bf16 = ml_dtypes.bfloat16
    query = np.asarray(query, dtype=np.float32)
    key = np.asarray(key, dtype=np.float32)
    value = np.asarray(value, dtype=np.float32)
    k_cache = np.asarray(k_cache, dtype=np.float32)
    v_cache = np.asarray(v_cache, dtype=np.float32)
    pos = np.asarray(cache_position, dtype=np.int64)
    pt = np.asarray(page_table, dtype=np.int64)

    budgets, assign, offs = _plan(pos)
    tot = int(sum(budgets))
    dtot = int(sum(n * G for n in budgets))
    nc = _get_nc(budgets)
    _COMPILED["last_nc"] = nc

    qg = (query.reshape(B, HKV, G, D) * SCALE).astype(bf16)

    in_maps = []
    for i in range(NCORES):
        kt = np.zeros(tot * D * S, dtype=bf16)
        vv = np.zeros(tot * S * D, dtype=bf16)
        qt = np.zeros((D, U * G), dtype=bf16)
        posv = np.zeros((S, U), dtype=np.float32)
        for j in range(U):
            nb = budgets[j]
            if nb == 0:
                continue
            uid = assign[j, i]
            b, h = divmod(int(uid), HKV)
            nv = int(-(-pos[b] // S))            # valid pages for this seq
            pages = pt[b][:nv]
            o0 = int(offs[j])
            kj = k_cache[pages, h].astype(bf16)  # [nv, S, D]
            vj = v_cache[pages, h].astype(bf16)
            ktj = np.zeros((D, nb * S), dtype=bf16)
            ktj[:, : nv * S] = kj.transpose(2, 0, 1).reshape(D, nv * S)
            vvj = np.zeros((S, nb * D), dtype=bf16)
            vvj[:, : nv * D] = vj.transpose(1, 0, 2).reshape(S, nv * D)
            kt[o0 * D * S : (o0 + nb) * D * S] = ktj.reshape(-1)
            vv[o0 * S * D : (o0 + nb) * S * D] = vvj.reshape(-1)
            qt[:, j * G : (j + 1) * G] = qg[b, h].T
            posv[:, j] = float(pos[b])
        in_maps.append({"qt": qt, "kt": kt, "v": vv, "posv": posv})

    _COMPILED["in_maps"] = in_maps
    res = run_bass_kernel_spmd(nc, in_maps, core_ids=list(range(NCORES)))
    outs = res.results

    out_bhg = np.zeros((B, HKV, G, D), dtype=np.float64)
    den_sum = np.zeros((B, HKV, G), dtype=np.float64)
    for i in range(NCORES):
        o = outs[i]["o"]          # [D, U*G]
        den = outs[i]["den"]      # [1, dtot]
        doff = 0
        for j in range(U):
            nb = budgets[j]
            if nb == 0:
                continue
            uid = assign[j, i]
            b, h = divmod(int(uid), HKV)
            out_bhg[b, h] = o[:, j * G : (j + 1) * G].T
            den_sum[b, h] = den[0, doff : doff + nb * G].reshape(nb, G).sum(0)
            doff += nb * G

    # new-token contribution (host rank-1 term)
    qgf = query.reshape(B, HKV, G, D)
    s_new = np.einsum("bkgd,bkd->bkg", qgf, key[:, :, 0, :]) * SCALE
    w_new = np.exp(s_new)                                       # [B, HKV, G]
    num = out_bhg + w_new[..., None] * value[:, :, 0, :][:, :, None, :]
    out = (num / (den_sum + w_new)[..., None]).reshape(B, H, 1, D)

    # cache update (host scatter)
    kc = np.array(k_cache)
    vc = np.array(v_cache)
    phys = pt[np.arange(B), pos // S]
    slot = pos % S
    kc[phys, :, slot, :] = key[:, :, 0, :]
    vc[phys, :, slot, :] = value[:, :, 0, :]

    return out.astype(np.float32), kc, vc
